# revision 5
# baseline (speedup 1.0000x reference)
"""DGCNN forward on 8 Trainium2 NeuronCores (Bass/Tile), data-parallel over batch.

kernel(**inputs) takes the FULL inputs from setup_inputs() and returns the
FULL [B, N, 1024] output.  Each core processes one point cloud end-to-end in
SBUF; training-mode BN stats are made exact across the batch with small
AllReduces between layers.

Self-contained: hardcodes B=8, D=60, N=1024, k=20 and the conv dims.
"""

import json
import os
from contextlib import ExitStack

import numpy as np

import concourse.bass as bass
import concourse.tile as tile
from concourse import mybir
from concourse.vector_clock import ScopedClock
from concourse.bass_utils import run_bass_kernel_spmd

F32 = mybir.dt.float32
F16 = mybir.dt.float16
U16 = mybir.dt.uint16

B, D, N, K = 8, 60, 1024, 20
NT = N // 128            # n-tiles per core
SPT = 128 * K            # samples per n-tile (2560)
S_TOTAL = N * K          # samples per core (20480)
EPS = 1e-5
ALPHA = 0.2
NEG_BIG = -1e30

C1, C2, C3, C4, C5, C6 = 64, 64, 128, 256, 256, 1024


# ---------------------------------------------------------------------------
# environment fixups (this walrus rejects instructions with >1 sync wait)
# ---------------------------------------------------------------------------

_FIX_COUNT = [0]


def _split_multiwaits(bir_json: bytes) -> bytes:
    m = json.loads(bir_json)
    changed = False
    for f in m.get("functions", []):
        for bb in f.get("blocks", f.get("basicblocks", [])):
            insts = bb.get("instructions")
            if not insts:
                continue
            out = []
            for ins in insts:
                si = ins.get("sync_info") or {}
                ow = si.get("on_wait") or []
                if len(ow) > 1:
                    changed = True
                    for w in ow[:-1]:
                        _FIX_COUNT[0] += 1
                        out.append({
                            "debug": ins.get("debug"),
                            "engine": ins["engine"],
                            "ins": [],
                            "name": f"I-waitfix-{_FIX_COUNT[0]}",
                            "opcode": "NoOp",
                            "outs": [],
                            "sync_info": {"on_update": [], "on_wait": [w]},
                        })
                    si["on_wait"] = [ow[-1]]
                out.append(ins)
            bb["instructions"] = out
    return json.dumps(m).encode() if changed else bir_json


def _install_fixups():
    import concourse.bass_utils as bu
    import concourse.bass2jax as b2j

    orig = bu.compile_bir_kernel
    if getattr(orig, "_waitfix_wrapped", False):
        return

    def wrapped(bir_json, tmpdir, neff_name="file.neff"):
        return orig(_split_multiwaits(bir_json), tmpdir, neff_name)

    wrapped._waitfix_wrapped = True
    bu.compile_bir_kernel = wrapped
    b2j.compile_bir_kernel = wrapped


class _TC(tile.TileContext):
    """TileContext whose exit drain carries at most one sync wait per inst."""

    def _drain_and_barrier(self, tick_clock, wait_clock):
        nop0 = self.nc.sync.nop(nofuse=True)
        wait_clock.add_sem_waits(nop0.ins, ScopedClock({None: tick_clock.global_clock}))
        si = nop0.ins.sync_info
        waits = list(si.on_wait) if si is not None and si.on_wait else []
        if len(waits) > 1:
            si.on_wait = [waits[0]]
            for w in waits[1:]:
                n = self.nc.sync.nop(nofuse=True)
                n.ins.sync_info = mybir.SyncInfo(on_wait=[w], on_update=[])
        self.nc.sync.drain()
        self.nc.all_engine_barrier()
        popped = self.nc._tile_sem_poison_stack.pop()
        assert popped is self._sem_poison
        self.nc.clear_and_free_semaphores(list(self.sems.allocated().values()))
        self.nc.all_engine_barrier()


# ---------------------------------------------------------------------------
# device program
# ---------------------------------------------------------------------------

def _build_program():
    nc = bass.Bass("TRN2", target_bir_lowering=False, debug=False, num_devices=8)

    xb = nc.dram_tensor("xb", [D, N], F32, kind="ExternalInput")
    w1aT = nc.dram_tensor("w1aT", [D, C1], F32, kind="ExternalInput")
    w1vT = nc.dram_tensor("w1vT", [D, C1], F32, kind="ExternalInput")
    w2T = nc.dram_tensor("w2T", [C1, C2], F16, kind="ExternalInput")
    w3T = nc.dram_tensor("w3T", [C2, C3], F16, kind="ExternalInput")
    w4T = nc.dram_tensor("w4T", [C3, C4], F16, kind="ExternalInput")
    w5T = nc.dram_tensor("w5T", [128, 4, C5], F16, kind="ExternalInput")
    w6T = nc.dram_tensor("w6T", [128, 2, C6], F16, kind="ExternalInput")
    gb1 = nc.dram_tensor("gb1", [C1, 2], F32, kind="ExternalInput")
    gb2 = nc.dram_tensor("gb2", [C2, 2], F32, kind="ExternalInput")
    gb3 = nc.dram_tensor("gb3", [C3, 2], F32, kind="ExternalInput")
    gb4 = nc.dram_tensor("gb4", [128, 4], F32, kind="ExternalInput")
    gb5 = nc.dram_tensor("gb5", [128, 4], F32, kind="ExternalInput")
    gb6 = nc.dram_tensor("gb6", [128, 16], F32, kind="ExternalInput")
    iota_in = nc.dram_tensor("iota_in", [128, NT], F32, kind="ExternalInput")
    sv_in = nc.dram_tensor("sv_in", [128, SPT], F16, kind="ExternalInput")

    out = nc.dram_tensor("o", [128, 8, N], F32, kind="ExternalOutput")
    debug = os.environ.get("DGCNN_DEBUG", "0") == "1"
    if debug:
        dbg_pd = nc.dram_tensor("dbg_pd", [128, N], F32, kind="ExternalOutput")
        dbg_idx = nc.dram_tensor("dbg_idx", [128, 24], U16, kind="ExternalOutput")
        dbg_idxrow = nc.dram_tensor("dbg_idxrow", [128, SPT], U16, kind="ExternalOutput")
        dbg_sm = nc.dram_tensor("dbg_sm", [128, SPT], F16, kind="ExternalOutput")
        dbg_y1 = nc.dram_tensor("dbg_y1", [C1, S_TOTAL], F16, kind="ExternalOutput")
        dbg_ut = nc.dram_tensor("dbg_ut", [128, NT, C1], F16, kind="ExternalOutput")
        dbg_vt = nc.dram_tensor("dbg_vt", [128, NT, C1], F16, kind="ExternalOutput")
        dbg_st1 = nc.dram_tensor("dbg_st1", [C1, 2], F32, kind="ExternalOutput")
        dbg_xc0 = nc.dram_tensor("dbg_xc0", [128, N], F16, kind="ExternalOutput")
        dbg_y2 = nc.dram_tensor("dbg_y2", [C2, S_TOTAL], F16, kind="ExternalOutput")
        dbg_xc = nc.dram_tensor("dbg_xc", [128, 4, N], F16, kind="ExternalOutput")
        dbg_y5 = nc.dram_tensor("dbg_y5", [128, 2, N], F32, kind="ExternalOutput")
        dbg_h5 = nc.dram_tensor("dbg_h5", [128, 2, N], F16, kind="ExternalOutput")
        dbg_y6 = nc.dram_tensor("dbg_y6", [128, 8, N], F32, kind="ExternalOutput")

    with _TC(nc) as tc, ExitStack() as ctx:
        sb = ctx.enter_context(tc.tile_pool(name="sb", bufs=1))
        work = ctx.enter_context(tc.tile_pool(name="work", bufs=2))
        spool = ctx.enter_context(tc.tile_pool(name="spool", bufs=2))
        ps = ctx.enter_context(tc.tile_pool(name="ps", bufs=2, space="PSUM"))
        psbig = ctx.enter_context(tc.tile_pool(name="psbig", bufs=1, space="PSUM"))
        ps1 = ctx.enter_context(tc.tile_pool(name="ps1", bufs=1, space="PSUM"))
        pdpool = ctx.enter_context(tc.tile_pool(name="pdpool", bufs=1))
        dram = ctx.enter_context(tc.tile_pool(name="dram", bufs=1, space="DRAM"))

        # ------------- load inputs -------------
        # x_a = [x; ones], x_b = [x; -xx/2]: key[n,m] = dot(x_a[:,n], x_b[:,m])
        #                                            = dot(x_n, x_m) - xx[m]/2
        x_a = sb.tile([D + 1, N], F32)
        nc.vector.memset(x_a[:], 1.0)            # row D stays all-ones
        nc.sync.dma_start(out=x_a[:D, :], in_=xb[:])
        x_b = sb.tile([D + 1, N], F32)
        nc.sync.dma_start(out=x_b[:D, :], in_=xb[:])
        w1a_sb = sb.tile([D, C1], F32)
        nc.sync.dma_start(out=w1a_sb[:], in_=w1aT[:])
        w1v_sb = sb.tile([D, C1], F32)
        nc.sync.dma_start(out=w1v_sb[:], in_=w1vT[:])
        w2_sb = sb.tile([C1, C2], F16)
        nc.sync.dma_start(out=w2_sb[:], in_=w2T[:])
        w3_sb = sb.tile([C2, C3], F16)
        nc.sync.dma_start(out=w3_sb[:], in_=w3T[:])
        w4_sb = sb.tile([C3, C4], F16)
        nc.sync.dma_start(out=w4_sb[:], in_=w4T[:])
        w5_sb = sb.tile([128, 4, C5], F16)
        nc.sync.dma_start(out=w5_sb[:], in_=w5T[:])
        w6_sb = sb.tile([128, 2, C6], F16)
        nc.sync.dma_start(out=w6_sb[:], in_=w6T[:])
        gb_sb = {}
        for name, t, c, g in (("1", gb1, C1, 1), ("2", gb2, C2, 1),
                              ("3", gb3, C3, 1), ("4", gb4, 128, 2),
                              ("5", gb5, 128, 2), ("6", gb6, 128, 8)):
            tt = sb.tile([c, 2 * g], F32, tag=f"gb{name}")
            nc.sync.dma_start(out=tt[:], in_=t[:])
            gb_sb[name] = tt
        iota_sb = sb.tile([128, NT], F32)
        nc.sync.dma_start(out=iota_sb[:], in_=iota_in[:])
        sv_sb = sb.tile([128, SPT], F16)
        nc.sync.dma_start(out=sv_sb[:], in_=sv_in[:])

        eps_col = sb.tile([128, 1], F32)
        nc.vector.memset(eps_col[:], EPS)
        ones_col = sb.tile([D, 1], F32)
        nc.vector.memset(ones_col[:], 1.0)

        # ------------- warmup collective (overlaps with KNN) -------------
        warm_in = dram.tile([16, 2], F32, tag="warm_in")
        warm_out = dram.tile([16, 2], F32, tag="warm_out")
        warm_sb = sb.tile([16, 2], F32)
        nc.vector.memset(warm_sb[:], 1.0)
        nc.sync.dma_start(out=warm_in[:], in_=warm_sb[:])
        nc.gpsimd.collective_compute(
            "AllReduce", mybir.AluOpType.add,
            replica_groups=[list(range(8))],
            ins=[warm_in.opt()], outs=[warm_out.opt()],
        )

        # ------------- residents -------------
        # big activations share slots by lifetime:
        #   resA: y1 (P0-P1) -> y3 (P2-P3) -> y6 (P5b-P6)
        #   resB: y2 (P1-P2) -> y4a (P3-P4) -> y5 (P5-P5b)
        #   resC: y4b (P3-P4)
        y1res = sb.tile([C1, S_TOTAL], F16, tag="resA")
        y2res = sb.tile([C2, S_TOTAL], F16, tag="resB")
        xc0 = sb.tile([128, N], F16)   # [x1; x2]
        xc1 = sb.tile([128, N], F16)   # x3
        xc2 = sb.tile([128, N], F16)   # x4[0:128]
        xc3 = sb.tile([128, N], F16)   # x4[128:256]
        x2tmp = sb.tile([C2, N], F16)
        h5 = sb.tile([128, 2, N], F16)
        uT = sb.tile([128, NT, C1], F16)
        vT = sb.tile([128, NT, C1], F16)

        idx_scr = dram.tile([NT, 1, 128 * K], U16, tag="idx_scr")

        # ------------- squared norms -> x_b row D -------------
        xsq = pdpool.tile([128, N], F32, tag="pd")
        nc.vector.tensor_mul(out=xsq[:D, :], in0=x_a[:D, :], in1=x_a[:D, :])
        xxrow = work.tile([1, N], F32, tag="xxrow")
        for h in range(2):
            pxx = ps1.tile([1, 512], F32, tag="paux")
            nc.tensor.matmul(out=pxx[:], lhsT=ones_col[:],
                             rhs=xsq[:D, 512 * h:512 * (h + 1)], start=True, stop=True)
            nc.scalar.mul(out=xxrow[:, 512 * h:512 * (h + 1)], in_=pxx[:], mul=-0.5)
        # partition-60 writes need a DMA (engines require 32-aligned bases)
        nc.sync.dma_start(out=x_b[D:D + 1, :], in_=xxrow[:])

        # ------------- uT / vT : uT[m, c] = sum_d x[d, m] w1a[c, d] -------------
        for t in range(NT):
            pu = ps1.tile([128, C1], F32, tag="paux")
            nc.tensor.matmul(out=pu[:], lhsT=x_a[:D, 128 * t:128 * (t + 1)],
                             rhs=w1a_sb[:], start=True, stop=True)
            nc.scalar.copy(out=uT[:, t, :], in_=pu[:])
            pv = ps1.tile([128, C1], F32, tag="paux")
            nc.tensor.matmul(out=pv[:], lhsT=x_a[:D, 128 * t:128 * (t + 1)],
                             rhs=w1v_sb[:], start=True, stop=True)
            nc.scalar.copy(out=vT[:, t, :], in_=pv[:])

        # ------------- P0: KNN + top-k + selection-matmul y1 + stats1 ----------
        st1 = sb.tile([C1, NT * 5, 6], F32)
        for t in range(NT):
            pd = pdpool.tile([128, N], F32, tag="pd")
            for h in range(2):
                pdp = ps1.tile([128, 512], F32, tag="paux")
                nc.tensor.matmul(out=pdp[:], lhsT=x_a[:, 128 * t:128 * (t + 1)],
                                 rhs=x_b[:, 512 * h:512 * (h + 1)], start=True, stop=True)
                nc.scalar.copy(out=pd[:, 512 * h:512 * (h + 1)], in_=pdp[:])
            pd2 = pdpool.tile([128, N], F32, tag="pd2")
            idx16 = work.tile([128, 24], U16, tag="idx16")
            cur = pd
            for r in range(3):
                mx = work.tile([128, 8], F32, tag="mx")
                nc.vector.max(out=mx[:], in_=cur[:])
                nc.vector.max_index(out=idx16[:, 8 * r:8 * r + 8], in_max=mx[:],
                                    in_values=cur[:])
                if r < 2:
                    dst = pd2 if r == 0 else pd
                    nc.vector.match_replace(out=dst[:], in_to_replace=mx[:],
                                            in_values=cur[:], imm_value=NEG_BIG)
                    cur = dst
            nc.sync.dma_start(
                out=idx_scr[t].rearrange("one (p j) -> (one p) j", j=K),
                in_=idx16[:, :K])
            idxrow = pdpool.tile([128, SPT], U16, tag="idxrow")
            nc.sync.dma_start(out=idxrow[:], in_=idx_scr[t].to_broadcast([128, SPT]))
            if debug and t == 0:
                nc.sync.dma_start(out=dbg_pd[:], in_=pd[:])
                nc.sync.dma_start(out=dbg_idx[:], in_=idx16[:])
                nc.sync.dma_start(out=dbg_idxrow[:], in_=idxrow[:])
            pyb = psbig.tile([C1, SPT], F32, tag="py1big")
            for mt in range(NT):
                sm = spool.tile([128, SPT], F16, tag="sm")
                nc.vector.tensor_scalar(
                    out=sm[:], in0=idxrow[:],
                    scalar1=iota_sb[:, mt:mt + 1], scalar2=None,
                    op0=mybir.AluOpType.is_equal,
                )
                if debug and t == 0 and mt == 0:
                    nc.sync.dma_start(out=dbg_sm[:], in_=sm[:])
                for c in range(5):
                    sl = slice(512 * c, 512 * (c + 1))
                    nc.tensor.matmul(out=pyb[:, sl], lhsT=uT[:, mt, :], rhs=sm[:, sl],
                                     start=(mt == 0), stop=False)
            for c in range(5):
                sl = slice(512 * c, 512 * (c + 1))
                nc.tensor.matmul(out=pyb[:, sl], lhsT=vT[:, t, :], rhs=sv_sb[:, sl],
                                 start=False, stop=True)
                nc.vector.bn_stats(out=st1[:, 5 * t + c, :], in_=pyb[:, sl])
                nc.scalar.copy(out=y1res[:, SPT * t + 512 * c:SPT * t + 512 * (c + 1)],
                               in_=pyb[:, sl])

        if debug:
            nc.sync.dma_start(out=dbg_y1[:], in_=y1res[:])
            nc.sync.dma_start(out=dbg_ut[:], in_=uT[:])
            nc.sync.dma_start(out=dbg_vt[:], in_=vT[:])

        # ------------- BN stat exchange helper -------------
        def allreduce_stats(name, stats_tile, c, groups):
            mv = work.tile([c, groups, 2], F32, tag=f"mv_{name}")
            for g in range(groups):
                src = stats_tile[:, g] if groups > 1 else stats_tile[:]
                nc.vector.bn_aggr(out=mv[:, g, :], in_=src)
            pay = work.tile([c, 2 * groups], F32, tag=f"pay_{name}")
            for g in range(groups):
                nc.vector.tensor_scalar_mul(pay[:, 2 * g:2 * g + 1], mv[:, g, 0:1], 0.125)
                m2 = work.tile([c, 1], F32, tag=f"m2_{name}")
                nc.vector.tensor_mul(out=m2[:], in0=mv[:, g, 0:1], in1=mv[:, g, 0:1])
                nc.vector.tensor_add(out=m2[:], in0=m2[:], in1=mv[:, g, 1:2])
                nc.vector.tensor_scalar_mul(pay[:, 2 * g + 1:2 * g + 2], m2[:], 0.125)
            b_in = dram.tile([c, 2 * groups], F32, tag=f"arin_{name}")
            b_out = dram.tile([c, 2 * groups], F32, tag=f"arout_{name}")
            nc.sync.dma_start(out=b_in[:], in_=pay[:])
            nc.gpsimd.collective_compute(
                "AllReduce", mybir.AluOpType.add,
                replica_groups=[list(range(8))],
                ins=[b_in.opt()], outs=[b_out.opt()],
            )
            red = work.tile([c, 2 * groups], F32, tag=f"red_{name}")
            nc.sync.dma_start(out=red[:], in_=b_out[:])
            st = sb.tile([c, groups, 2], F32, tag=f"st_{name}")
            gbt = gb_sb[name]
            for g in range(groups):
                var = work.tile([c, 1], F32, tag=f"var_{name}")
                nc.vector.tensor_mul(out=var[:], in0=red[:, 2 * g:2 * g + 1],
                                     in1=red[:, 2 * g:2 * g + 1])
                nc.vector.tensor_sub(out=var[:], in0=red[:, 2 * g + 1:2 * g + 2], in1=var[:])
                std = work.tile([c, 1], F32, tag=f"std_{name}")
                nc.scalar.activation(out=std[:], in_=var[:],
                                     func=mybir.ActivationFunctionType.Sqrt,
                                     bias=eps_col[:c, :], scale=1.0)
                nc.vector.reciprocal(out=std[:], in_=std[:])
                nc.vector.tensor_mul(out=st[:, g, 0:1], in0=gbt[:, 2 * g:2 * g + 1],
                                     in1=std[:])
                tmp = work.tile([c, 1], F32, tag=f"tmp_{name}")
                nc.vector.tensor_mul(out=tmp[:], in0=red[:, 2 * g:2 * g + 1],
                                     in1=st[:, g, 0:1])
                nc.vector.tensor_sub(out=st[:, g, 1:2], in0=gbt[:, 2 * g + 1:2 * g + 2],
                                     in1=tmp[:])
            return st

        st_l1 = allreduce_stats("1", st1, C1, 1)

        # ------------- P1: h1 -> x1, conv2 + stats2 -------------
        st2 = sb.tile([C2, NT * 5, 6], F32)
        for t in range(NT):
            h1 = work.tile([C1, SPT], F16, tag="hbuf")
            nc.scalar.activation(out=h1[:], in_=y1res[:, SPT * t:SPT * (t + 1)],
                                 func=mybir.ActivationFunctionType.Prelu,
                                 bias=st_l1[:, 0, 1:2], scale=st_l1[:, 0, 0:1], alpha=ALPHA)
            nc.vector.tensor_reduce(
                out=xc0[0:C1, 128 * t:128 * (t + 1)].rearrange("c (n one) -> c n one", one=1),
                in_=h1[:].rearrange("c (n k) -> c n k", k=K),
                axis=mybir.AxisListType.X, op=mybir.AluOpType.max)
            for c in range(5):
                py = ps.tile([C2, 512], F32, tag="pmm")
                nc.tensor.matmul(out=py[:], lhsT=w2_sb[:], rhs=h1[:, 512 * c:512 * (c + 1)],
                                 start=True, stop=True)
                nc.vector.bn_stats(out=st2[:, 5 * t + c, :], in_=py[:])
                nc.scalar.copy(out=y2res[:, SPT * t + 512 * c:SPT * t + 512 * (c + 1)],
                               in_=py[:])

        if debug:
            nc.sync.dma_start(out=dbg_st1[:], in_=st_l1[:, 0, :])
            nc.sync.dma_start(out=dbg_xc0[:], in_=xc0[:])
            nc.sync.dma_start(out=dbg_y2[:], in_=y2res[:])
        st_l2 = allreduce_stats("2", st2, C2, 1)

        # ------------- P2: h2 -> x2, conv3 + stats3 -------------
        y3res = sb.tile([C3, S_TOTAL], F16, tag="resA")
        st3 = sb.tile([C3, NT * 5, 6], F32)
        for t in range(NT):
            h2 = work.tile([C2, SPT], F16, tag="hbuf")
            nc.scalar.activation(out=h2[:], in_=y2res[:, SPT * t:SPT * (t + 1)],
                                 func=mybir.ActivationFunctionType.Prelu,
                                 bias=st_l2[:, 0, 1:2], scale=st_l2[:, 0, 0:1], alpha=ALPHA)
            nc.vector.tensor_reduce(
                out=x2tmp[:, 128 * t:128 * (t + 1)].rearrange("c (n one) -> c n one", one=1),
                in_=h2[:].rearrange("c (n k) -> c n k", k=K),
                axis=mybir.AxisListType.X, op=mybir.AluOpType.max)
            for c in range(5):
                py = ps.tile([C3, 512], F32, tag="pmm")
                nc.tensor.matmul(out=py[:], lhsT=w3_sb[:], rhs=h2[:, 512 * c:512 * (c + 1)],
                                 start=True, stop=True)
                nc.vector.bn_stats(out=st3[:, 5 * t + c, :], in_=py[:])
                nc.scalar.copy(out=y3res[:, SPT * t + 512 * c:SPT * t + 512 * (c + 1)],
                               in_=py[:])
        nc.sync.dma_start(out=xc0[C1:128, :], in_=x2tmp[:])

        st_l3 = allreduce_stats("3", st3, C3, 1)

        # ------------- P3: h3 -> x3, conv4 + stats4 -------------
        y4res_a = sb.tile([128, S_TOTAL], F16, tag="resB")
        y4res_b = sb.tile([128, S_TOTAL], F16, tag="resC")
        st4 = sb.tile([128, 2, NT * 5, 6], F32)
        for t in range(NT):
            h3 = work.tile([C3, SPT], F16, tag="hbuf")
            nc.scalar.activation(out=h3[:], in_=y3res[:, SPT * t:SPT * (t + 1)],
                                 func=mybir.ActivationFunctionType.Prelu,
                                 bias=st_l3[:, 0, 1:2], scale=st_l3[:, 0, 0:1], alpha=ALPHA)
            nc.vector.tensor_reduce(
                out=xc1[:, 128 * t:128 * (t + 1)].rearrange("c (n one) -> c n one", one=1),
                in_=h3[:].rearrange("c (n k) -> c n k", k=K),
                axis=mybir.AxisListType.X, op=mybir.AluOpType.max)
            for c in range(5):
                sl = slice(512 * c, 512 * (c + 1))
                res_sl = slice(SPT * t + 512 * c, SPT * t + 512 * (c + 1))
                for g, dst in ((0, y4res_a), (1, y4res_b)):
                    py = ps.tile([128, 512], F32, tag="pmm")
                    nc.tensor.matmul(out=py[:], lhsT=w4_sb[:, 128 * g:128 * (g + 1)],
                                     rhs=h3[:, sl], start=True, stop=True)
                    nc.vector.bn_stats(out=st4[:, g, 5 * t + c, :], in_=py[:])
                    nc.scalar.copy(out=dst[:, res_sl], in_=py[:])

        st_l4 = allreduce_stats("4", st4, 128, 2)

        if debug:
            nc.sync.dma_start(out=dbg_xc[:, 0, :], in_=xc0[:])

        # ------------- P4: h4 -> x4 -------------
        for t in range(NT):
            for g, src, dst in ((0, y4res_a, xc2), (1, y4res_b, xc3)):
                h4 = work.tile([128, SPT], F16, tag="hbuf")
                nc.scalar.activation(out=h4[:], in_=src[:, SPT * t:SPT * (t + 1)],
                                     func=mybir.ActivationFunctionType.Prelu,
                                     bias=st_l4[:, g, 1:2], scale=st_l4[:, g, 0:1],
                                     alpha=ALPHA)
                nc.vector.tensor_reduce(
                    out=dst[:, 128 * t:128 * (t + 1)].rearrange("c (n one) -> c n one", one=1),
                    in_=h4[:].rearrange("c (n k) -> c n k", k=K),
                    axis=mybir.AxisListType.X, op=mybir.AluOpType.max)

        # ------------- P5: conv5 + stats5 -------------
        y5res = sb.tile([128, 2, N], F32, tag="resB")
        st5 = sb.tile([128, 2, 2, 6], F32)
        for half in range(2):
            nsl = slice(512 * half, 512 * (half + 1))
            for g in range(2):
                py = ps.tile([128, 512], F32, tag="pmm")
                for ki, kt in enumerate((xc0, xc1, xc2, xc3)):
                    nc.tensor.matmul(out=py[:],
                                     lhsT=w5_sb[:, ki, 128 * g:128 * (g + 1)],
                                     rhs=kt[:, nsl], start=(ki == 0), stop=(ki == 3))
                nc.vector.bn_stats(out=st5[:, g, half, :], in_=py[:])
                nc.vector.tensor_copy(out=y5res[:, g, nsl], in_=py[:])

        st_l5 = allreduce_stats("5", st5, 128, 2)

        if debug:
            nc.sync.dma_start(out=dbg_xc[:, 1, :], in_=xc1[:])
            nc.sync.dma_start(out=dbg_xc[:, 2, :], in_=xc2[:])
            nc.sync.dma_start(out=dbg_xc[:, 3, :], in_=xc3[:])
            nc.sync.dma_start(out=dbg_y5[:], in_=y5res[:])

        # ------------- P5b: h5 -> conv6 + stats6 -------------
        y6res = sb.tile([128, 8, N], F32, tag="resA")
        st6 = sb.tile([128, 8, 2, 6], F32)
        for g in range(2):
            nc.scalar.activation(out=h5[:, g, :], in_=y5res[:, g, :],
                                 func=mybir.ActivationFunctionType.Prelu,
                                 bias=st_l5[:, g, 1:2], scale=st_l5[:, g, 0:1], alpha=ALPHA)
        for half in range(2):
            nsl = slice(512 * half, 512 * (half + 1))
            for g in range(8):
                py = ps.tile([128, 512], F32, tag="pmm")
                for ki in range(2):
                    nc.tensor.matmul(out=py[:],
                                     lhsT=w6_sb[:, ki, 128 * g:128 * (g + 1)],
                                     rhs=h5[:, ki, nsl], start=(ki == 0), stop=(ki == 1))
                nc.vector.bn_stats(out=st6[:, g, half, :], in_=py[:])
                nc.vector.tensor_copy(out=y6res[:, g, nsl], in_=py[:])

        st_l6 = allreduce_stats("6", st6, 128, 8)

        if debug:
            nc.sync.dma_start(out=dbg_h5[:], in_=h5[:])
            nc.sync.dma_start(out=dbg_y6[:], in_=y6res[:])

        # ------------- P6: h6 -> output (in-place on y6res) -------------
        for g in range(8):
            nc.scalar.activation(out=y6res[:, g, :], in_=y6res[:, g, :],
                                 func=mybir.ActivationFunctionType.Prelu,
                                 bias=st_l6[:, g, 1:2], scale=st_l6[:, g, 0:1], alpha=ALPHA)
        nc.sync.dma_start(out=out[:], in_=y6res[:])

    return nc


_CACHED = {}


def _get_program():
    if "nc" not in _CACHED:
        _install_fixups()
        _CACHED["nc"] = _build_program()
    return _CACHED["nc"]


def kernel(**inputs) -> np.ndarray:
    nc = _get_program()

    x = np.asarray(inputs["x"], np.float32)          # [B, D, N]
    w1 = np.asarray(inputs["w1"], np.float32)        # [64, 120]
    w1a = w1[:, :D]
    w1b = w1[:, D:]
    w1aT = np.ascontiguousarray(w1a.T)               # [60, 64]
    w1vT = np.ascontiguousarray((w1b - w1a).T)       # [60, 64]

    def wt16(name):
        return np.ascontiguousarray(
            np.asarray(inputs[name], np.float32).T).astype(np.float16)

    w2T, w3T, w4T = wt16("w2"), wt16("w3"), wt16("w4")
    w5T = np.ascontiguousarray(wt16("w5").reshape(4, 128, C5).transpose(1, 0, 2))
    w6T = np.ascontiguousarray(wt16("w6").reshape(2, 128, C6).transpose(1, 0, 2))

    def gbpack(gi, bei, c, groups):
        g = np.asarray(inputs[gi], np.float32).reshape(groups, -1)
        be = np.asarray(inputs[bei], np.float32).reshape(groups, -1)
        rows = c if groups == 1 else 128
        outp = np.zeros((rows, 2 * groups), np.float32)
        for gr in range(groups):
            outp[:, 2 * gr] = g[gr]
            outp[:, 2 * gr + 1] = be[gr]
        return outp

    gbs = dict(
        gb1=gbpack("g1", "be1", C1, 1), gb2=gbpack("g2", "be2", C2, 1),
        gb3=gbpack("g3", "be3", C3, 1), gb4=gbpack("g4", "be4", C4, 2),
        gb5=gbpack("g5", "be5", C5, 2), gb6=gbpack("g6", "be6", C6, 8),
    )

    iota = (np.arange(128)[:, None] + 128 * np.arange(NT)[None, :]).astype(np.float32)
    sv = np.zeros((128, SPT), np.float16)
    for p in range(128):
        sv[p, K * p:K * (p + 1)] = 1.0

    common = dict(w1aT=w1aT, w1vT=w1vT, w2T=w2T, w3T=w3T, w4T=w4T, w5T=w5T, w6T=w6T,
                  iota_in=iota, sv_in=sv, **gbs)
    in_maps = [dict(common, xb=np.ascontiguousarray(x[b])) for b in range(B)]

    trace = os.environ.get("DGCNN_TRACE", "0") == "1"
    res = run_bass_kernel_spmd(nc, in_maps, core_ids=list(range(8)), trace=trace)
    _CACHED["last_results"] = res

    outs = []
    for b in range(B):
        o = res.results[b]["o"]                      # [128, 8, N]
        h6 = o.transpose(1, 0, 2).reshape(C6, N)     # [1024 ch, N pts]
        outs.append(h6.T)                            # [N, 1024]
    return np.stack(outs, axis=0).astype(np.float32)



# revision 17
# speedup vs baseline: 1.1554x; 1.1554x over previous
"""DGCNN forward on 8 Trainium2 NeuronCores (Bass/Tile), data-parallel over batch.

kernel(**inputs) takes the FULL inputs from setup_inputs() and returns the
FULL [B, N, 1024] output.  Each core processes one point cloud end-to-end in
SBUF; training-mode BN stats are made exact across the batch with small
AllReduces between layers.

Self-contained: hardcodes B=8, D=60, N=1024, k=20 and the conv dims.
"""

import json
import os
from contextlib import ExitStack

import numpy as np

import concourse.bass as bass
import concourse.tile as tile
from concourse import mybir
from concourse.vector_clock import ScopedClock
from concourse.bass_utils import run_bass_kernel_spmd

F32 = mybir.dt.float32
F16 = mybir.dt.float16
U16 = mybir.dt.uint16

B, D, N, K = 8, 60, 1024, 20
NT = N // 128            # n-tiles per core
SPT = 128 * K            # samples per n-tile (2560)
S_TOTAL = N * K          # samples per core (20480)
EPS = 1e-5
ALPHA = 0.2
NEG_BIG = -1e30

C1, C2, C3, C4, C5, C6 = 64, 64, 128, 256, 256, 1024

AF = mybir.ActivationFunctionType
ALU = mybir.AluOpType
AX = mybir.AxisListType


# ---------------------------------------------------------------------------
# environment fixups (this walrus rejects instructions with >1 sync wait)
# ---------------------------------------------------------------------------

_FIX_COUNT = [0]


def _split_multiwaits(bir_json: bytes) -> bytes:
    m = json.loads(bir_json)
    changed = False
    for f in m.get("functions", []):
        for bb in f.get("blocks", f.get("basicblocks", [])):
            insts = bb.get("instructions")
            if not insts:
                continue
            out = []
            for ins in insts:
                si = ins.get("sync_info") or {}
                ow = si.get("on_wait") or []
                if len(ow) > 1:
                    changed = True
                    for w in ow[:-1]:
                        _FIX_COUNT[0] += 1
                        out.append({
                            "debug": ins.get("debug"),
                            "engine": ins["engine"],
                            "ins": [],
                            "name": f"I-waitfix-{_FIX_COUNT[0]}",
                            "opcode": "NoOp",
                            "outs": [],
                            "sync_info": {"on_update": [], "on_wait": [w]},
                        })
                    si["on_wait"] = [ow[-1]]
                out.append(ins)
            bb["instructions"] = out
    return json.dumps(m).encode() if changed else bir_json


def _install_fixups():
    import concourse.bass_utils as bu
    import concourse.bass2jax as b2j

    orig = bu.compile_bir_kernel
    if getattr(orig, "_waitfix_wrapped", False):
        return

    def wrapped(bir_json, tmpdir, neff_name="file.neff"):
        return orig(_split_multiwaits(bir_json), tmpdir, neff_name)

    wrapped._waitfix_wrapped = True
    bu.compile_bir_kernel = wrapped
    b2j.compile_bir_kernel = wrapped


class _TC(tile.TileContext):
    """TileContext whose exit drain carries at most one sync wait per inst."""

    def _drain_and_barrier(self, tick_clock, wait_clock):
        nop0 = self.nc.sync.nop(nofuse=True)
        wait_clock.add_sem_waits(nop0.ins, ScopedClock({None: tick_clock.global_clock}))
        si = nop0.ins.sync_info
        waits = list(si.on_wait) if si is not None and si.on_wait else []
        if len(waits) > 1:
            si.on_wait = [waits[0]]
            for w in waits[1:]:
                n = self.nc.sync.nop(nofuse=True)
                n.ins.sync_info = mybir.SyncInfo(on_wait=[w], on_update=[])
        self.nc.sync.drain()
        self.nc.all_engine_barrier()
        popped = self.nc._tile_sem_poison_stack.pop()
        assert popped is self._sem_poison
        self.nc.clear_and_free_semaphores(list(self.sems.allocated().values()))
        self.nc.all_engine_barrier()


# ---------------------------------------------------------------------------
# device program
# ---------------------------------------------------------------------------

def _build_program():
    nc = bass.Bass("TRN2", target_bir_lowering=False, debug=False, num_devices=8)

    xb = nc.dram_tensor("xb", [D, N], F32, kind="ExternalInput")
    w1aT = nc.dram_tensor("w1aT", [D, C1], F32, kind="ExternalInput")
    w1vT = nc.dram_tensor("w1vT", [D, C1], F32, kind="ExternalInput")
    w2T = nc.dram_tensor("w2T", [C1, C2], F16, kind="ExternalInput")
    w3T = nc.dram_tensor("w3T", [C2, C3], F16, kind="ExternalInput")
    w4T = nc.dram_tensor("w4T", [C3, C4], F16, kind="ExternalInput")
    w5T = nc.dram_tensor("w5T", [128, 4, C5], F16, kind="ExternalInput")
    w6T = nc.dram_tensor("w6T", [128, 2, C6], F16, kind="ExternalInput")
    # gb{i}: [c, G, 2] with [..., 0] = gamma, [..., 1] = beta
    gb1 = nc.dram_tensor("gb1", [C1, 1, 2], F32, kind="ExternalInput")
    gb2 = nc.dram_tensor("gb2", [C2, 1, 2], F32, kind="ExternalInput")
    gb3 = nc.dram_tensor("gb3", [C3, 1, 2], F32, kind="ExternalInput")
    gb4 = nc.dram_tensor("gb4", [128, 2, 2], F32, kind="ExternalInput")
    gb5 = nc.dram_tensor("gb5", [128, 2, 2], F32, kind="ExternalInput")
    gb6 = nc.dram_tensor("gb6", [128, 8, 2], F32, kind="ExternalInput")
    iota_in = nc.dram_tensor("iota_in", [128, NT], F32, kind="ExternalInput")
    biasq_in = nc.dram_tensor("biasq_in", [128, NT], F32, kind="ExternalInput")
    sv_in = nc.dram_tensor("sv_in", [128, SPT], F16, kind="ExternalInput")

    out = nc.dram_tensor("o", [128, 8, N], F32, kind="ExternalOutput")

    with _TC(nc) as tc, ExitStack() as ctx:
        sb = ctx.enter_context(tc.tile_pool(name="sb", bufs=1))
        work = ctx.enter_context(tc.tile_pool(name="work", bufs=2))
        work1 = ctx.enter_context(tc.tile_pool(name="work1", bufs=1))
        spool = ctx.enter_context(tc.tile_pool(name="spool", bufs=2))
        ps = ctx.enter_context(tc.tile_pool(name="ps", bufs=1, space="PSUM"))
        aux = ctx.enter_context(tc.tile_pool(name="aux", bufs=2, space="PSUM"))
        dram = ctx.enter_context(tc.tile_pool(name="dram", bufs=1, space="DRAM"))

        # ------------- load inputs -------------
        # x_a = [x; ones], x_b = [x; -xx/2]: key[n,m] = dot(x_a[:,n], x_b[:,m])
        #                                            = dot(x_n, x_m) - xx[m]/2
        x_a = sb.tile([D + 1, N], F32)
        nc.vector.memset(x_a[:], 1.0)            # row D stays all-ones
        nc.sync.dma_start(out=x_a[:D, :], in_=xb[:])
        x_b = sb.tile([D + 1, N], F32)
        nc.sync.dma_start(out=x_b[:D, :], in_=xb[:])
        w1a_sb = sb.tile([D, C1], F32)
        nc.sync.dma_start(out=w1a_sb[:], in_=w1aT[:])
        w1v_sb = sb.tile([D, C1], F32)
        nc.sync.dma_start(out=w1v_sb[:], in_=w1vT[:])
        w2_sb = sb.tile([C1, C2], F16)
        nc.sync.dma_start(out=w2_sb[:], in_=w2T[:])
        w3_sb = sb.tile([C2, C3], F16)
        nc.sync.dma_start(out=w3_sb[:], in_=w3T[:])
        w4_sb = sb.tile([C3, C4], F16)
        nc.sync.dma_start(out=w4_sb[:], in_=w4T[:])
        w5_sb = sb.tile([128, 4, C5], F16)
        nc.sync.dma_start(out=w5_sb[:], in_=w5T[:])
        w6_sb = sb.tile([128, 2, C6], F16)
        nc.sync.dma_start(out=w6_sb[:], in_=w6T[:])
        gb_sb = {}
        for name, t, c, g in (("1", gb1, C1, 1), ("2", gb2, C2, 1),
                              ("3", gb3, C3, 1), ("4", gb4, 128, 2),
                              ("5", gb5, 128, 2), ("6", gb6, 128, 8)):
            tt = sb.tile([c, g, 2], F32, tag=f"gb{name}")
            nc.sync.dma_start(out=tt[:], in_=t[:])
            gb_sb[name] = tt
        iota_sb = sb.tile([128, NT], F32)
        nc.sync.dma_start(out=iota_sb[:], in_=iota_in[:])
        biasq_sb = sb.tile([128, NT], F32)
        nc.sync.dma_start(out=biasq_sb[:], in_=biasq_in[:])
        sv_sb = sb.tile([128, SPT], F16)
        nc.sync.dma_start(out=sv_sb[:], in_=sv_in[:])

        eps_col = sb.tile([128, 1], F32)
        nc.vector.memset(eps_col[:], EPS)
        one_col = sb.tile([128, 1], F32)
        nc.vector.memset(one_col[:], 1.0)
        ones_col = sb.tile([D, 1], F32)
        nc.vector.memset(ones_col[:], 1.0)

        # ------------- warmup collectives (overlap with P0) -------------
        warm_in = dram.tile([16, 2], F32, tag="warm_in")
        warm_mid = dram.tile([16, 2], F32, tag="warm_mid")
        warm_out = dram.tile([16, 2], F32, tag="warm_out")
        warm_sb = sb.tile([16, 2], F32)
        nc.vector.memset(warm_sb[:], 1.0)
        nc.sync.dma_start(out=warm_in[:], in_=warm_sb[:])
        nc.gpsimd.collective_compute(
            "AllReduce", ALU.add, replica_groups=[list(range(8))],
            ins=[warm_in.opt()], outs=[warm_mid.opt()],
        )
        nc.gpsimd.collective_compute(
            "AllReduce", ALU.add, replica_groups=[list(range(8))],
            ins=[warm_mid.opt()], outs=[warm_out.opt()],
        )

        # ------------- residents -------------
        # big activations share slots by lifetime:
        #   resA: y1 (P0-P1) -> y3 (P2-P3) -> y6 (P5b-P6)
        #   resB: y2 (P1-P2) -> y4a (P3-P4) -> out staging (P6)
        #   resC: y4b (P3-P4) -> out staging (P6)
        y1res = sb.tile([C1, S_TOTAL], F16, tag="resA")
        y2res = sb.tile([C2, S_TOTAL], F16, tag="resB")
        uT = sb.tile([128, NT, C1], F16)
        vT = sb.tile([128, NT, C1], F16)
        xc0 = sb.tile([128, N], F16)      # [x1; x2] (pre-act until AR, then in-place)
        xc1 = sb.tile([128, N], F16)      # x3
        xc2 = sb.tile([128, N], F16)      # x4[0:128]
        xc3 = sb.tile([128, N], F16)      # x4[128:256]
        y5res = sb.tile([128, 2, N], F16)  # conv5 out; prelu'd in place after AR5

        st1 = sb.tile([C1, 1, NT * 5, 6], F32)
        st2 = sb.tile([C2, 1, NT * 5, 6], F32)
        st3 = sb.tile([C3, 1, NT * 5, 6], F32)
        st4 = sb.tile([128, 2, NT * 5, 6], F32)
        st5 = sb.tile([128, 2, 2, 6], F32)
        st6 = sb.tile([128, 8, 2, 6], F32)

        idx_scr = dram.tile([NT, 1, 128 * K], U16, tag="idx_scr")

        # ------------- squared norms -> x_b row D -------------
        xsq = work1.tile([D, N], F32, tag="scratch4k")
        nc.vector.tensor_mul(out=xsq[:], in0=x_a[:D, :], in1=x_a[:D, :])
        xxrow = work1.tile([1, N], F32, tag="xxrow")
        for h in range(2):
            pxx = aux.tile([1, 512], F32, tag="aux")
            nc.tensor.matmul(out=pxx[:], lhsT=ones_col[:],
                             rhs=xsq[:, 512 * h:512 * (h + 1)], start=True, stop=True)
            nc.scalar.mul(out=xxrow[:, 512 * h:512 * (h + 1)], in_=pxx[:], mul=-0.5)
        # partition-60 writes need a DMA (engines require 32-aligned bases)
        nc.sync.dma_start(out=x_b[D:D + 1, :], in_=xxrow[:])

        # ------------- uT / vT : uT[m, c] = sum_d x[d, m] w1a[c, d] -------------
        for t in range(NT):
            pu = aux.tile([128, C1], F32, tag="aux")
            nc.tensor.matmul(out=pu[:], lhsT=x_a[:D, 128 * t:128 * (t + 1)],
                             rhs=w1a_sb[:], start=True, stop=True)
            nc.scalar.copy(out=uT[:, t, :], in_=pu[:])
            pv = aux.tile([128, C1], F32, tag="aux")
            nc.tensor.matmul(out=pv[:], lhsT=x_a[:D, 128 * t:128 * (t + 1)],
                             rhs=w1v_sb[:], start=True, stop=True)
            nc.scalar.copy(out=vT[:, t, :], in_=pv[:])

        # ------------- helpers -------------
        def emit_pd(t):
            """pairwise-key matmuls for tile t -> pd sbuf tile (work pool)."""
            pd = work.tile([128, N], F32, tag="pd")
            for h in range(2):
                pdp = aux.tile([128, 512], F32, tag="aux")
                nc.tensor.matmul(out=pdp[:], lhsT=x_a[:, 128 * t:128 * (t + 1)],
                                 rhs=x_b[:, 512 * h:512 * (h + 1)], start=True, stop=True)
                nc.scalar.copy(out=pd[:, 512 * h:512 * (h + 1)], in_=pdp[:])
            return pd

        def emit_stats(layer_st, yres, g, t):
            """bn_stats for tile t of a [c, S_TOTAL] f16 resident (vector)."""
            for ci in range(5):
                sl = slice(SPT * t + 512 * ci, SPT * t + 512 * (ci + 1))
                nc.vector.bn_stats(out=layer_st[:, g, 5 * t + ci, :], in_=yres[:, sl])

        def emit_tree(yres, c, t, out_ap):
            """max over k=20 for tile t: 2 gpsimd tensor_max + 1 vector reduce."""
            v = yres[:, SPT * t:SPT * (t + 1)].rearrange("c (n k) -> c n k", k=K)
            t1 = work1.tile([c, 128, 10], F16, tag="tr1")
            nc.vector.tensor_max(out=t1[:], in0=v[:, :, 0:10], in1=v[:, :, 10:20])
            t2 = work1.tile([c, 128, 5], F16, tag="tr2")
            nc.vector.tensor_max(out=t2[:], in0=t1[:, :, 0:5], in1=t1[:, :, 5:10])
            nc.vector.tensor_reduce(out=out_ap, in_=t2[:], axis=AX.X, op=ALU.max)

        def allreduce_stats(name, st_raw, c, G):
            """exact-batch BN: aggregate local stats, AllReduce -> [c,G,2] scale/shift."""
            mv = work.tile([c, G, 2], F32, tag="mv")
            for g in range(G):
                nc.vector.bn_aggr(out=mv[:, g, :], in_=st_raw[:, g])
            pay = work.tile([c, G, 2], F32, tag="pay")
            nc.vector.tensor_scalar_mul(pay[:, :, 0:1], mv[:, :, 0:1], 0.125)
            m2 = work.tile([c, G, 1], F32, tag="m2")
            nc.vector.tensor_mul(out=m2[:], in0=mv[:, :, 0:1], in1=mv[:, :, 0:1])
            nc.vector.tensor_add(out=m2[:], in0=m2[:], in1=mv[:, :, 1:2])
            nc.vector.tensor_scalar_mul(pay[:, :, 1:2], m2[:], 0.125)
            b_in = dram.tile([c, G, 2], F32, tag=f"arin_{name}")
            b_out = dram.tile([c, G, 2], F32, tag=f"arout_{name}")
            nc.sync.dma_start(out=b_in[:], in_=pay[:])
            nc.gpsimd.collective_compute(
                "AllReduce", ALU.add, replica_groups=[list(range(8))],
                ins=[b_in.opt()], outs=[b_out.opt()],
            )
            red = work.tile([c, G, 2], F32, tag="red")
            nc.sync.dma_start(out=red[:], in_=b_out[:])
            var = work.tile([c, G, 1], F32, tag="var")
            nc.vector.tensor_mul(out=var[:], in0=red[:, :, 0:1], in1=red[:, :, 0:1])
            nc.vector.tensor_sub(out=var[:], in0=red[:, :, 1:2], in1=var[:])
            inv = work.tile([c, G, 1], F32, tag="inv")
            nc.scalar.activation(out=inv[:], in_=var[:], func=AF.Sqrt,
                                 bias=eps_col[:c, :], scale=1.0)
            nc.vector.reciprocal(out=inv[:], in_=inv[:])
            st = sb.tile([c, G, 2], F32, tag=f"st_{name}")
            gbt = gb_sb[name]
            nc.vector.tensor_mul(out=st[:, :, 0:1], in0=gbt[:, :, 0:1], in1=inv[:])
            tmp = work.tile([c, G, 1], F32, tag="tmp")
            nc.vector.tensor_mul(out=tmp[:], in0=red[:, :, 0:1], in1=st[:, :, 0:1])
            nc.vector.tensor_sub(out=st[:, :, 1:2], in0=gbt[:, :, 1:2], in1=tmp[:])
            return st

        # ------------- P0: KNN + top-k + selection-matmul y1 -------------
        # sm engine schedule: 3 vector / 3 scalar / 2 gpsimd per tile
        SM_ENG = ["v", "s", "v", "s", "v", "s", "v", "s"]

        pd_cur = emit_pd(0)
        for t in range(NT):
            # prefetch next tile's pairwise keys (tensor+scalar, overlaps topk)
            pd_next = emit_pd(t + 1) if t + 1 < NT else None

            # top-24 via 3 rounds of max8/max_index (+2 match_replace)
            pd = pd_cur
            pd2 = work1.tile([128, N], F32, tag="scratch4k")
            idx16 = work.tile([128, 24], U16, tag="idx16")
            cur = pd
            for r in range(3):
                mx = work.tile([128, 8], F32, tag="mx")
                nc.vector.max(out=mx[:], in_=cur[:])
                nc.vector.max_index(out=idx16[:, 8 * r:8 * r + 8], in_max=mx[:],
                                    in_values=cur[:])
                if r < 2:
                    dst = pd2 if r == 0 else pd
                    nc.vector.match_replace(out=dst[:], in_to_replace=mx[:],
                                            in_values=cur[:], imm_value=NEG_BIG)
                    cur = dst
            nc.sync.dma_start(
                out=idx_scr[t].rearrange("one (p j) -> (one p) j", j=K),
                in_=idx16[:, :K])
            idxrow = work.tile([128, SPT], U16, tag="idxrow", bufs=1)
            nc.sync.dma_start(out=idxrow[:], in_=idx_scr[t].to_broadcast([128, SPT]))

            # selection matmuls: accumulate over the 8 m-tiles into 5 psum chunks
            psc = [ps.tile([C1, 512], F32, tag=f"pmm{c}", name=f"psc{c}")
                   for c in range(5)]
            for mt in range(NT):
                sm = spool.tile([128, SPT], F16, tag="sm")
                eng = SM_ENG[mt]
                if eng == "v":
                    nc.vector.tensor_scalar(
                        out=sm[:], in0=idxrow[:],
                        scalar1=iota_sb[:, mt:mt + 1], scalar2=None, op0=ALU.is_equal)
                else:
                    # one-hot on the scalar engine: relu(1 - 1024*((idx - tgt)/32)^2)
                    a1 = work.tile([128, SPT], F16, tag="big2560")
                    nc.scalar.activation(out=a1[:], in_=idxrow[:], func=AF.Square,
                                         bias=biasq_sb[:, mt:mt + 1], scale=1.0 / 32.0)
                    nc.scalar.activation(out=sm[:], in_=a1[:], func=AF.Relu,
                                         bias=one_col[:], scale=-1024.0)
                for c in range(5):
                    sl = slice(512 * c, 512 * (c + 1))
                    nc.tensor.matmul(out=psc[c][:], lhsT=uT[:, mt, :], rhs=sm[:, sl],
                                     start=(mt == 0), stop=False)
            for c in range(5):
                sl = slice(512 * c, 512 * (c + 1))
                res_sl = slice(SPT * t + 512 * c, SPT * t + 512 * (c + 1))
                nc.tensor.matmul(out=psc[c][:], lhsT=vT[:, t, :], rhs=sv_sb[:, sl],
                                 start=False, stop=True)
                nc.scalar.copy(out=y1res[:, res_sl], in_=psc[c][:])

            # software-pipelined stats + max-k for the previous tile
            if t > 0:
                emit_stats(st1, y1res, 0, t - 1)
                emit_tree(y1res, C1, t - 1, xc0[0:C1, 128 * (t - 1):128 * t])
            pd_cur = pd_next
        emit_stats(st1, y1res, 0, NT - 1)
        emit_tree(y1res, C1, NT - 1, xc0[0:C1, 128 * (NT - 1):])

        st_l1 = allreduce_stats("1", st1, C1, 1)
        nc.scalar.activation(out=xc0[0:C1, :], in_=xc0[0:C1, :], func=AF.Prelu,
                             bias=st_l1[:, 0, 1:2], scale=st_l1[:, 0, 0:1], alpha=ALPHA)

        # ------------- generic conv-over-samples phase -------------
        def conv_phase(src, dsts, w_sb, st_in, st_out, cin, gw):
            """dsts[g][:, s] = w[g] @ prelu(st_in * src[:, s]) ; yields tile idx
            for the caller to attach per-tile max-k reductions (pipelined)."""
            groups = len(dsts)
            for t in range(NT):
                h = work.tile([cin, SPT], F16, tag="big2560")
                nc.scalar.activation(out=h[:], in_=src[:, SPT * t:SPT * (t + 1)],
                                     func=AF.Prelu, bias=st_in[:, 0, 1:2],
                                     scale=st_in[:, 0, 0:1], alpha=ALPHA)
                for g in range(groups):
                    for c in range(5):
                        sl = slice(512 * c, 512 * (c + 1))
                        res_sl = slice(SPT * t + 512 * c, SPT * t + 512 * (c + 1))
                        py = ps.tile([gw, 512], F32, tag=f"pmm{c}")
                        nc.tensor.matmul(out=py[:], lhsT=w_sb[:, gw * g:gw * (g + 1)],
                                         rhs=h[:, sl], start=True, stop=True)
                        if c < 3:
                            nc.scalar.copy(out=dsts[g][:, res_sl], in_=py[:])
                        else:
                            nc.vector.tensor_copy(out=dsts[g][:, res_sl], in_=py[:])
                if t > 0:
                    for g in range(groups):
                        emit_stats(st_out, dsts[g], g, t - 1)
                    yield t - 1
            for g in range(groups):
                emit_stats(st_out, dsts[g], g, NT - 1)
            yield NT - 1

        # ------------- P1: h1 -> conv2 -------------
        x2tmp = work1.tile([C2, N], F16, tag="scratch4k")  # pd2 slot is free now
        for t in conv_phase(y1res, [y2res], w2_sb, st_l1, st2, C1, C2):
            emit_tree(y2res, C2, t, x2tmp[:, 128 * t:128 * (t + 1)])
        st_l2 = allreduce_stats("2", st2, C2, 1)
        nc.scalar.activation(out=x2tmp[:], in_=x2tmp[:], func=AF.Prelu,
                             bias=st_l2[:, 0, 1:2], scale=st_l2[:, 0, 0:1], alpha=ALPHA)
        nc.sync.dma_start(out=xc0[C1:128, :], in_=x2tmp[:])

        # ------------- P2: h2 -> conv3 -------------
        y3res = sb.tile([C3, S_TOTAL], F16, tag="resA")
        for t in conv_phase(y2res, [y3res], w3_sb, st_l2, st3, C2, C3):
            emit_tree(y3res, C3, t, xc1[:, 128 * t:128 * (t + 1)])
        st_l3 = allreduce_stats("3", st3, C3, 1)
        nc.scalar.activation(out=xc1[:], in_=xc1[:], func=AF.Prelu,
                             bias=st_l3[:, 0, 1:2], scale=st_l3[:, 0, 0:1], alpha=ALPHA)

        # ------------- P3: h3 -> conv4 (2 channel groups) -------------
        y4res_a = sb.tile([128, S_TOTAL], F16, tag="resB")
        y4res_b = sb.tile([128, S_TOTAL], F16, tag="resC")
        for t in conv_phase(y3res, [y4res_a, y4res_b], w4_sb, st_l3, st4, C3, 128):
            emit_tree(y4res_a, 128, t, xc2[:, 128 * t:128 * (t + 1)])
            emit_tree(y4res_b, 128, t, xc3[:, 128 * t:128 * (t + 1)])
        st_l4 = allreduce_stats("4", st4, 128, 2)

        # ------------- P4: x4 = prelu(st4 * max_k y4) in place -------------
        nc.scalar.activation(out=xc2[:], in_=xc2[:], func=AF.Prelu,
                             bias=st_l4[:, 0, 1:2], scale=st_l4[:, 0, 0:1], alpha=ALPHA)
        nc.scalar.activation(out=xc3[:], in_=xc3[:], func=AF.Prelu,
                             bias=st_l4[:, 1, 1:2], scale=st_l4[:, 1, 0:1], alpha=ALPHA)

        # ------------- P5: conv5 -------------
        for half in range(2):
            nsl = slice(512 * half, 512 * (half + 1))
            for g in range(2):
                py = ps.tile([128, 512], F32, tag=f"pmm{2 * g + half}")
                for ki, kt in enumerate((xc0, xc1, xc2, xc3)):
                    nc.tensor.matmul(out=py[:],
                                     lhsT=w5_sb[:, ki, 128 * g:128 * (g + 1)],
                                     rhs=kt[:, nsl], start=(ki == 0), stop=(ki == 3))
                nc.scalar.copy(out=y5res[:, g, nsl], in_=py[:])
                nc.vector.bn_stats(out=st5[:, g, half, :], in_=y5res[:, g, nsl])
        st_l5 = allreduce_stats("5", st5, 128, 2)

        # ------------- P5b: h5 (in place) -> conv6 -------------
        y6res = sb.tile([128, 8, N], F16, tag="resA")
        for g in range(2):
            nc.scalar.activation(out=y5res[:, g, :], in_=y5res[:, g, :],
                                 func=AF.Prelu, bias=st_l5[:, g, 1:2],
                                 scale=st_l5[:, g, 0:1], alpha=ALPHA)
        for half in range(2):
            nsl = slice(512 * half, 512 * (half + 1))
            for g in range(8):
                py = ps.tile([128, 512], F32, tag=f"pmm{g % 4}")
                for ki in range(2):
                    nc.tensor.matmul(out=py[:],
                                     lhsT=w6_sb[:, ki, 128 * g:128 * (g + 1)],
                                     rhs=y5res[:, ki, nsl], start=(ki == 0), stop=(ki == 1))
                if g % 2 == 0:
                    nc.scalar.copy(out=y6res[:, g, nsl], in_=py[:])
                else:
                    nc.vector.tensor_copy(out=y6res[:, g, nsl], in_=py[:])
                nc.vector.bn_stats(out=st6[:, g, half, :], in_=y6res[:, g, nsl])
        st_l6 = allreduce_stats("6", st6, 128, 8)

        # ------------- P6: h6 -> output, streamed per channel group -------------
        # staging reuses the resB/resC slots (y4 is dead by now)
        for g in range(8):
            stage = sb.tile([128, N], F32, tag=("resB" if g % 2 == 0 else "resC"))
            nc.scalar.activation(out=stage[:], in_=y6res[:, g, :],
                                 func=AF.Prelu, bias=st_l6[:, g, 1:2],
                                 scale=st_l6[:, g, 0:1], alpha=ALPHA)
            nc.sync.dma_start(out=out[:, g, :], in_=stage[:])

    return nc


_CACHED = {}


def _get_program():
    if "nc" not in _CACHED:
        _install_fixups()
        _CACHED["nc"] = _build_program()
    return _CACHED["nc"]


def kernel(**inputs) -> np.ndarray:
    nc = _get_program()

    x = np.asarray(inputs["x"], np.float32)          # [B, D, N]
    w1 = np.asarray(inputs["w1"], np.float32)        # [64, 120]
    w1a = w1[:, :D]
    w1b = w1[:, D:]
    w1aT = np.ascontiguousarray(w1a.T)               # [60, 64]
    w1vT = np.ascontiguousarray((w1b - w1a).T)       # [60, 64]

    def wt16(name):
        return np.ascontiguousarray(
            np.asarray(inputs[name], np.float32).T).astype(np.float16)

    w2T, w3T, w4T = wt16("w2"), wt16("w3"), wt16("w4")
    w5T = np.ascontiguousarray(wt16("w5").reshape(4, 128, C5).transpose(1, 0, 2))
    w6T = np.ascontiguousarray(wt16("w6").reshape(2, 128, C6).transpose(1, 0, 2))

    def gbpack(gi, bei, c, groups):
        g = np.asarray(inputs[gi], np.float32).reshape(groups, -1)
        be = np.asarray(inputs[bei], np.float32).reshape(groups, -1)
        rows = c if groups == 1 else 128
        outp = np.zeros((rows, groups, 2), np.float32)
        for gr in range(groups):
            outp[:, gr, 0] = g[gr]
            outp[:, gr, 1] = be[gr]
        return outp

    gbs = dict(
        gb1=gbpack("g1", "be1", C1, 1), gb2=gbpack("g2", "be2", C2, 1),
        gb3=gbpack("g3", "be3", C3, 1), gb4=gbpack("g4", "be4", C4, 2),
        gb5=gbpack("g5", "be5", C5, 2), gb6=gbpack("g6", "be6", C6, 8),
    )

    iota = (np.arange(128)[:, None] + 128 * np.arange(NT)[None, :]).astype(np.float32)
    biasq = (-(np.arange(128)[:, None] + 128 * np.arange(NT)[None, :]) / 32.0
             ).astype(np.float32)
    sv = np.zeros((128, SPT), np.float16)
    for p in range(128):
        sv[p, K * p:K * (p + 1)] = 1.0

    common = dict(w1aT=w1aT, w1vT=w1vT, w2T=w2T, w3T=w3T, w4T=w4T, w5T=w5T, w6T=w6T,
                  iota_in=iota, biasq_in=biasq, sv_in=sv, **gbs)
    in_maps = [dict(common, xb=np.ascontiguousarray(x[b])) for b in range(B)]

    trace = os.environ.get("DGCNN_TRACE", "0") == "1"
    res = run_bass_kernel_spmd(nc, in_maps, core_ids=list(range(8)), trace=trace)
    _CACHED["last_results"] = res

    outs = []
    for b in range(B):
        o = res.results[b]["o"]                      # [128, 8, N]
        h6 = o.transpose(1, 0, 2).reshape(C6, N)     # [1024 ch, N pts]
        outs.append(h6.T)                            # [N, 1024]
    return np.stack(outs, axis=0).astype(np.float32)
